# revision 16
# baseline (speedup 1.0000x reference)
"""GQA attention kernel for 8 trn2 NeuronCores.

Reference computation (per head h of 32, d_k=64, S=2048, D=2048):
  q = split_heads(query @ wq); k = split_heads(key @ wk); v = split_heads(value @ wv)
  q,k = rope(q), rope(k);  kv heads repeated 4x (GQA groups)
  scores = q k^T / 8 (+ mask*-1e9); attn = softmax(scores); ctx = attn @ v
  out = merge_heads(ctx) @ wo;  returns (out, attn)

Sharding: tensor-parallel over heads. Core c handles q-heads [4c,4c+4) and
kv-head c. wq/wk/wv column-sharded, wo row-sharded; partial outputs summed
on host; attn shards concatenated on host. No collectives.

On-core dataflow (all activations kept transposed, [feature, seq]):
  qT/kT/vT streamed in; q'T = rope(wq_cT @ ...) via PE matmuls + PE
  permutation-matmul for rotate_half; scores per head via K=64 f32r
  matmuls; exp+rowsum fused on ScalarE (scale=1/8); normalize on ScalarE;
  attn^T via bf16 PE transpose; ctx^T = v-stationary matmuls; out =
  ctx^T-stationary matmuls vs wo. bf16 keeps PE at 1 cycle/row.
"""

import sys
import numpy as np

sys.path.insert(0, "/opt/trn_rl_repo")

import concourse.bass as bass
import concourse.mybir as mybir
import concourse.tile as tile
from concourse import bacc
from concourse.masks import make_identity

F32 = mybir.dt.float32
BF16 = mybir.dt.bfloat16
F32R = mybir.dt.float32r
AF = mybir.ActivationFunctionType

N_CORES = 8
D_MODEL = 2048
NUM_HEADS = 32
NUM_KV_HEADS = 8
D_K = 64
H_CORE = NUM_HEADS // N_CORES          # 4 q heads per core
SEQ = 2048


def r(x):
    return x


def build_kernel(S=SEQ, D=D_MODEL, use_mask=False):
    """Emit the per-core Bass program. S = sequence len, D = model dim."""
    nc = bacc.Bacc(None, target_bir_lowering=False)
    EC = H_CORE * D_K                  # 256 q-proj cols per core
    ND = D // 128                      # d-model chunks
    NS = S // 128                      # seq tiles
    JW = min(1024, S)                  # scores psum width (banks*512)
    NJH = S // JW                      # psum tiles per i-tile row
    IW = min(512, S)                   # i-slice width (ctx moving N)
    NISL = S // IW                     # i-slices per head
    NT = IW // 128                     # i-tiles per slice
    NJB = S // 128                     # j blocks (transpose granularity)

    qT = nc.dram_tensor("qT", [D, S], BF16, kind="ExternalInput")
    kT = nc.dram_tensor("kT", [D, S], BF16, kind="ExternalInput")
    vT = nc.dram_tensor("vT", [D, S], BF16, kind="ExternalInput")
    wq = nc.dram_tensor("wq", [D, EC], BF16, kind="ExternalInput")
    wk = nc.dram_tensor("wk", [D, D_K], BF16, kind="ExternalInput")
    wv = nc.dram_tensor("wv", [D, D_K], BF16, kind="ExternalInput")
    wo = nc.dram_tensor("wo", [2, 128, D], BF16, kind="ExternalInput")
    cosT = nc.dram_tensor("cosT", [128, S], F32, kind="ExternalInput")
    sinT = nc.dram_tensor("sinT", [128, S], F32, kind="ExternalInput")
    permT = nc.dram_tensor("permT", [128, 128], F32, kind="ExternalInput")
    if use_mask:
        maskv = nc.dram_tensor("maskv", [S, S], F32, kind="ExternalInput")
    attn4 = nc.dram_tensor("attn4", [H_CORE, S, S], F32, kind="ExternalOutput")
    outp = nc.dram_tensor("outp", [S, D], F32, kind="ExternalOutput")

    with tile.TileContext(nc) as tc:
        # ---- persistent tiles ----
        const_pool = tc.tile_pool(name="const", bufs=1)
        qp_pool = tc.tile_pool(name="qp", bufs=1)
        with const_pool as cp, qp_pool as qpp:
            cos_sb = cp.tile([128, S], F32)
            sin_sb = cp.tile([128, S], F32)
            perm_sb = cp.tile([128, 128], F32)
            ident = cp.tile([128, 128], F32)
            identb = cp.tile([128, 128], BF16)
            nc.scalar.dma_start(cos_sb[:, :], cosT[:, :])
            nc.scalar.dma_start(sin_sb[:, :], sinT[:, :])
            nc.scalar.dma_start(perm_sb[:, :], permT[:, :])
            make_identity(nc, ident[:, :])
            make_identity(nc, identb[:, :])

            wk_sb = cp.tile([128, ND * D_K], BF16)
            wv_sb = cp.tile([128, ND * D_K], BF16)
            for d in range(ND):
                nc.scalar.dma_start(wk_sb[:, d * D_K:(d + 1) * D_K],
                                  wk[d * 128:(d + 1) * 128, :])
                nc.scalar.dma_start(wv_sb[:, d * D_K:(d + 1) * D_K],
                                  wv[d * 128:(d + 1) * 128, :])

            # q'/k' (post-rope, transposed [c, s]) and v ([s, c]) and ctx^T
            qp_sb = [qpp.tile([128, S], BF16, tag=f"qp{e}", name=f"qp{e}")
                     for e in range(EC // 128)]
            kp_sb = qpp.tile([128, S], BF16)       # k' duplicated both halves
            v_sb = qpp.tile([128, NS * D_K], BF16)  # column block b = s-tile b
            ctxT_sb = [qpp.tile([128, S], BF16, tag=f"ctxT{e}", name=f"ctxT{e}")
                       for e in range(EC // 128)]

            # ================= Phase A: projections + rope =================
            with tc.tile_pool(name="wqp", bufs=1) as wqp, \
                 tc.tile_pool(name="stream", bufs=4) as stp, \
                 tc.tile_pool(name="araw", bufs=1) as arp, \
                 tc.tile_pool(name="scr", bufs=2) as scp, \
                 tc.tile_pool(name="psA", bufs=8, space="PSUM") as psA:
                wq_sb = wqp.tile([128, ND * EC], BF16)
                for d in range(ND):
                    nc.scalar.dma_start(wq_sb[:, d * EC:(d + 1) * EC],
                                      wq[d * 128:(d + 1) * 128, :])

                # ---- k projection: k_raw [64, S] ----
                k_raw = arp.tile([64, S], F32, tag="araw")
                psk = [psA.tile([64, 512], F32, tag="psA", name=f"psk{i}")
                       for i in range(S // 512)]
                kt_tiles = []
                for d in range(ND):
                    t = stp.tile([128, S], BF16, tag="stream")
                    nc.sync.dma_start(t[:, :], kT[d * 128:(d + 1) * 128, :])
                    kt_tiles.append(t)
                    for s4 in range(S // 512):
                        nc.tensor.matmul(
                            psk[s4][:, :],
                            r(wk_sb[:, d * D_K:(d + 1) * D_K]),
                            r(t[:, s4 * 512:(s4 + 1) * 512]),
                            start=(d == 0), stop=(d == ND - 1))
                for s4 in range(S // 512):
                    nc.vector.tensor_copy(k_raw[:, s4 * 512:(s4 + 1) * 512],
                                          psk[s4][:, :])
                # rope k into kp_sb[0:64], then duplicate to [64:128]
                for sl in range(S // 512):
                    sli = slice(sl * 512, (sl + 1) * 512)
                    rp = psA.tile([64, 512], F32, tag="psA")
                    nc.tensor.matmul(rp[:, :], perm_sb[0:64, 0:64],
                                     k_raw[:, sli], start=True, stop=True)
                    t1 = scp.tile([64, 512], F32, tag="scrk")
                    nc.vector.tensor_mul(t1[:, :], rp[:, :], sin_sb[0:64, sli])
                    t2 = scp.tile([64, 512], F32, tag="scrk2")
                    nc.vector.tensor_mul(t2[:, :], k_raw[:, sli], cos_sb[0:64, sli])
                    nc.vector.tensor_add(kp_sb[0:64, sli], t1[:, :], t2[:, :])
                nc.vector.tensor_copy(kp_sb[64:128, :], kp_sb[0:64, :])

                # ---- v projection: v_projT [64, S], then transpose to
                # v_sb [s-tile partitions, (b, c)] ----
                v_rawT = arp.tile([64, S], F32, tag="vrawT")
                psv = [psA.tile([64, 512], F32, tag="psA", name=f"psv{i}")
                       for i in range(S // 512)]
                for d in range(ND):
                    t = stp.tile([128, S], BF16, tag="stream")
                    nc.sync.dma_start(t[:, :], vT[d * 128:(d + 1) * 128, :])
                    for s4 in range(S // 512):
                        nc.tensor.matmul(
                            psv[s4][:, :],
                            r(wv_sb[:, d * D_K:(d + 1) * D_K]),
                            r(t[:, s4 * 512:(s4 + 1) * 512]),
                            start=(d == 0), stop=(d == ND - 1))
                for s4 in range(S // 512):
                    nc.vector.tensor_copy(v_rawT[:, s4 * 512:(s4 + 1) * 512],
                                          psv[s4][:, :])
                for g in range((NS * D_K + 511) // 512):
                    pvt = psA.tile([128, 512], F32, tag="psA")
                    nb = min(8, NS - g * 8)
                    for bb in range(nb):
                        b = g * 8 + bb
                        nc.tensor.transpose(
                            r(pvt[:, bb * D_K:(bb + 1) * D_K]),
                            r(v_rawT[:, b * 128:(b + 1) * 128]),
                            r(ident[0:64, 0:64]))
                    nc.vector.tensor_copy(
                        v_sb[:, g * 512:g * 512 + nb * D_K],
                        pvt[:, 0:nb * D_K])

                # ---- q projection: q_raw [e-tile][128, S] ----
                q_raw = [arp.tile([128, S], F32, tag=f"qraw{e}", name=f"qraw{e}")
                         for e in range(EC // 128)]
                psq = [[psA.tile([128, 512], F32, tag="psA", name=f"psq{e}_{i}")
                        for i in range(S // 512)] for e in range(EC // 128)]
                for d in range(ND):
                    t = stp.tile([128, S], BF16, tag="stream")
                    nc.sync.dma_start(t[:, :], qT[d * 128:(d + 1) * 128, :])
                    for e in range(EC // 128):
                        for s4 in range(S // 512):
                            nc.tensor.matmul(
                                psq[e][s4][:, :],
                                r(wq_sb[:, d * EC + e * 128:d * EC + (e + 1) * 128]),
                                r(t[:, s4 * 512:(s4 + 1) * 512]),
                                start=(d == 0), stop=(d == ND - 1))
                for e in range(EC // 128):
                    for s4 in range(S // 512):
                        nc.vector.tensor_copy(
                            q_raw[e][:, s4 * 512:(s4 + 1) * 512], psq[e][s4][:, :])
                # rope q
                for e in range(EC // 128):
                    for sl in range(S // 512):
                        sli = slice(sl * 512, (sl + 1) * 512)
                        rp = psA.tile([128, 512], F32, tag="psA")
                        nc.tensor.matmul(rp[:, :], perm_sb[:, :],
                                         q_raw[e][:, sli], start=True, stop=True)
                        t1 = scp.tile([128, 512], F32, tag="scrq")
                        nc.vector.tensor_mul(t1[:, :], rp[:, :], sin_sb[:, sli])
                        t2 = scp.tile([128, 512], F32, tag="scrq2")
                        nc.vector.tensor_mul(t2[:, :], q_raw[e][:, sli],
                                             cos_sb[:, sli])
                        nc.vector.tensor_add(qp_sb[e][:, sli], t1[:, :], t2[:, :])

            # ============ Phase B: scores / softmax / attn / ctx ============
            with tc.tile_pool(name="Epool", bufs=3) as ep, \
                 tc.tile_pool(name="Apool", bufs=NT + 1) as ap_, \
                 tc.tile_pool(name="Abpool", bufs=2 * NT + 1) as abp, \
                 tc.tile_pool(name="ETpool", bufs=2) as etp, \
                 tc.tile_pool(name="rpool", bufs=2 * NT) as rp_, \
                 tc.tile_pool(name="mpool", bufs=2) as mp, \
                 tc.tile_pool(name="psS", bufs=3, space="PSUM") as psS, \
                 tc.tile_pool(name="psC", bufs=2, space="PSUM") as psC:

                def transposes_and_ctx(h, isl, at_tiles):
                    # attn[i-slice, :] -> ET [j, (jb, i-slice)] via xbar DMA
                    # transpose, then ctx^T [64, IW] = sum_jb v_b^T . ET_b
                    pb = (h % 2) * 64
                    et = etp.tile([128, NJB, 128 * NT], BF16, tag="ET")
                    for t in range(NT):
                        nc.sync.dma_start_transpose(
                            et[:, :, t * 128:(t + 1) * 128], at_tiles[t][:, :])
                    pc = psC.tile([64, IW], F32, tag="psC")
                    for jb in range(NJB):
                        nc.tensor.matmul(
                            pc[:, :],
                            r(v_sb[:, jb * D_K:(jb + 1) * D_K]),
                            r(et[:, jb, :]),
                            start=(jb == 0), stop=(jb == NJB - 1))
                    nc.vector.tensor_copy(
                        ctxT_sb[h // 2][pb:pb + 64, isl * IW:(isl + 1) * IW],
                        pc[:, :])

                pending = None
                for h in range(H_CORE):
                    qp = qp_sb[h // 2]
                    pb = (h % 2) * 64
                    for isl in range(NISL):
                        at_tiles = []
                        for t in range(NT):
                            i0 = isl * IW + t * 128
                            pss = [psS.tile([128, JW], F32, tag="psS", name=f"pss{jh}")
                                   for jh in range(NJH)]
                            for jh in range(NJH):
                                for jq in range(JW // 512):
                                    j0 = jh * JW + jq * 512
                                    nc.tensor.matmul(
                                        pss[jh][:, jq * 512:(jq + 1) * 512],
                                        r(qp[pb:pb + 64, i0:i0 + 128]),
                                        r(kp_sb[pb:pb + 64, j0:j0 + 512]),
                                        start=True, stop=True)
                            if use_mask:
                                mt = mp.tile([128, S], F32, tag="mask")
                                nc.sync.dma_start(mt[:, :], maskv[i0:i0 + 128, :])
                                for jh in range(NJH):
                                    nc.vector.tensor_add(
                                        pss[jh][:, :], pss[jh][:, :],
                                        mt[:, jh * JW:(jh + 1) * JW])
                            Et = ep.tile([128, S], F32, tag="E")
                            racc = rp_.tile([128, NJH + 2], F32, tag="racc")
                            for jh in range(NJH):
                                nc.scalar.activation(
                                    Et[:, jh * JW:(jh + 1) * JW], pss[jh][:, :],
                                    AF.Exp, scale=0.125,
                                    accum_out=racc[:, jh:jh + 1])
                            # rowsum -> reciprocal
                            if NJH > 1:
                                nc.vector.tensor_add(
                                    racc[:, NJH:NJH + 1], racc[:, 0:1], racc[:, 1:2])
                                rsum = racc[:, NJH:NJH + 1]
                            else:
                                rsum = racc[:, 0:1]
                            nc.vector.reciprocal(racc[:, NJH + 1:NJH + 2], rsum)
                            At = ap_.tile([128, S], F32, tag="A")
                            Atb = abp.tile([128, S], BF16, tag="Ab")
                            # balance the two post-softmax passes across
                            # ScalarE/VectorE (ScalarE also owns exp)
                            if t % 2 == 0:
                                nc.vector.tensor_scalar_mul(
                                    At[:, :], Et[:, :],
                                    racc[:, NJH + 1:NJH + 2])
                                nc.scalar.activation(
                                    Atb[:, :], Et[:, :], AF.Copy,
                                    scale=racc[:, NJH + 1:NJH + 2])
                            else:
                                nc.scalar.activation(
                                    At[:, :], Et[:, :], AF.Copy,
                                    scale=racc[:, NJH + 1:NJH + 2])
                                nc.vector.tensor_scalar_mul(
                                    Atb[:, :], Et[:, :],
                                    racc[:, NJH + 1:NJH + 2])
                            nc.gpsimd.dma_start(attn4[h, i0:i0 + 128, :], At[:, :])
                            at_tiles.append(Atb)
                        if pending is not None:
                            transposes_and_ctx(*pending)
                        pending = (h, isl, at_tiles)
                if pending is not None:
                    transposes_and_ctx(*pending)

            # ================= Phase C: output projection =================
            with tc.tile_pool(name="wop", bufs=1) as wop, \
                 tc.tile_pool(name="opool", bufs=3) as op_, \
                 tc.tile_pool(name="psO", bufs=4, space="PSUM") as psO:
                wo_sb = [wop.tile([128, D], BF16, tag=f"wo{ch}", name=f"wo{ch}")
                         for ch in range(2)]
                for ch in range(2):
                    nc.sync.dma_start(wo_sb[ch][:, :], wo[ch, :, :])
                DW = min(512, D)
                for st in range(NS):
                    ot = op_.tile([128, D], F32, tag="out")
                    for dq in range(D // DW):
                        po = psO.tile([128, DW], F32, tag="psO")
                        for ch in range(2):
                            nc.tensor.matmul(
                                po[:, :],
                                r(ctxT_sb[ch][:, st * 128:(st + 1) * 128]),
                                r(wo_sb[ch][:, dq * DW:(dq + 1) * DW]),
                                start=(ch == 0), stop=(ch == 1))
                        nc.vector.tensor_copy(ot[:, dq * DW:(dq + 1) * DW], po[:, :])
                    nc.gpsimd.dma_start(outp[st * 128:(st + 1) * 128, :], ot[:, :])

    nc.compile()
    return nc


def _rope_tables(S):
    theta = 1.0 / (10000.0 ** (np.arange(0, D_K, 2, dtype=np.float32) / D_K))
    pos = np.arange(S, dtype=np.float32)
    freqs = np.outer(pos, theta)                       # (S, 32)
    cos = np.repeat(np.cos(freqs), 2, axis=-1)         # (S, 64)
    sin = np.repeat(np.sin(freqs), 2, axis=-1)
    cosT = np.ascontiguousarray(cos.T)                 # (64, S)
    sinT = np.ascontiguousarray(sin.T)
    return (np.tile(cosT, (2, 1)).astype(np.float32),
            np.tile(sinT, (2, 1)).astype(np.float32))  # (128, S)


def _perm_matrix():
    # lhsT for rotate_half: rot = P @ q, PT[c', c] = P[c, c']
    PT = np.zeros((64, 64), dtype=np.float32)
    idx = np.arange(32)
    PT[idx, idx + 32] = 1.0      # rot[c>=32] = q[c-32]
    PT[idx + 32, idx] = -1.0     # rot[c<32]  = -q[c+32]
    out = np.zeros((128, 128), dtype=np.float32)
    out[0:64, 0:64] = PT
    out[64:128, 64:128] = PT
    return out


_NC_CACHE = {}


def _get_nc(S, D, use_mask):
    key = (S, D, use_mask)
    if key not in _NC_CACHE:
        _NC_CACHE[key] = build_kernel(S, D, use_mask)
    return _NC_CACHE[key]


def kernel(query, key, value, mask, wq, wk, wv, wo, _trace=False):
    from concourse import bass_utils

    B, S, D = query.shape
    assert B == 1
    use_mask = bool(np.any(mask))

    nc = _get_nc(S, D, use_mask)

    import ml_dtypes
    bf16 = ml_dtypes.bfloat16
    qT = np.ascontiguousarray(np.asarray(query, np.float32)[0].T).astype(bf16)
    kT = np.ascontiguousarray(np.asarray(key, np.float32)[0].T).astype(bf16)
    vT = np.ascontiguousarray(np.asarray(value, np.float32)[0].T).astype(bf16)
    wq = np.asarray(wq, np.float32).astype(bf16)
    wk = np.asarray(wk, np.float32).astype(bf16)
    wv = np.asarray(wv, np.float32).astype(bf16)
    wo = np.asarray(wo, np.float32).astype(bf16)
    cosT, sinT = _rope_tables(S)
    permT = _perm_matrix()
    EC = H_CORE * D_K

    in_maps = []
    for c in range(N_CORES):
        m = {
            "qT": qT, "kT": kT, "vT": vT,
            "wq": np.ascontiguousarray(wq[:, c * EC:(c + 1) * EC]),
            "wk": np.ascontiguousarray(wk[:, c * D_K:(c + 1) * D_K]),
            "wv": np.ascontiguousarray(wv[:, c * D_K:(c + 1) * D_K]),
            "wo": np.ascontiguousarray(
                wo[c * EC:(c + 1) * EC, :].reshape(2, 128, D)),
            "cosT": cosT, "sinT": sinT, "permT": permT,
        }
        if use_mask:
            m["maskv"] = np.ascontiguousarray(
                (np.asarray(mask, np.float32)[0, 0] * np.float32(-8e9)))
        in_maps.append(m)

    res = bass_utils.run_bass_kernel_spmd(
        nc, in_maps, core_ids=list(range(N_CORES)), trace=_trace)

    out = np.zeros((S, D), dtype=np.float32)
    attn = np.empty((1, NUM_HEADS, S, S), dtype=np.float32)
    for c in range(N_CORES):
        out += res.results[c]["outp"]
        attn[0, c * H_CORE:(c + 1) * H_CORE] = res.results[c]["attn4"]
    if _trace:
        kernel._last_results = res
    return out[None], attn


# revision 18
# speedup vs baseline: 1.3115x; 1.3115x over previous
"""GQA attention kernel for 8 trn2 NeuronCores.

Reference computation (per head h of 32, d_k=64, S=2048, D=2048):
  q = split_heads(query @ wq); k = split_heads(key @ wk); v = split_heads(value @ wv)
  q,k = rope(q), rope(k);  kv heads repeated 4x (GQA groups)
  scores = q k^T / 8 (+ mask*-1e9); attn = softmax(scores); ctx = attn @ v
  out = merge_heads(ctx) @ wo;  returns (out, attn)

Sharding: tensor-parallel over heads. Core c handles q-heads [4c,4c+4) and
kv-head c. wq/wk/wv column-sharded, wo row-sharded; partial outputs summed
on host; attn shards concatenated on host. No collectives.

On-core dataflow (all activations kept transposed, [feature, seq]):
  qT/kT/vT streamed in; q'T = rope(wq_cT @ ...) via PE matmuls + PE
  permutation-matmul for rotate_half; scores per head via K=64 f32r
  matmuls; exp+rowsum fused on ScalarE (scale=1/8); normalize on ScalarE;
  attn^T via bf16 PE transpose; ctx^T = v-stationary matmuls; out =
  ctx^T-stationary matmuls vs wo. bf16 keeps PE at 1 cycle/row.
"""

import sys
import numpy as np

sys.path.insert(0, "/opt/trn_rl_repo")

import concourse.bass as bass
import concourse.mybir as mybir
import concourse.tile as tile
from concourse import bacc
from concourse.masks import make_identity

F32 = mybir.dt.float32
BF16 = mybir.dt.bfloat16
F32R = mybir.dt.float32r
AF = mybir.ActivationFunctionType

N_CORES = 8
D_MODEL = 2048
NUM_HEADS = 32
NUM_KV_HEADS = 8
D_K = 64
H_CORE = NUM_HEADS // N_CORES          # 4 q heads per core
SEQ = 2048


def r(x):
    return x


def build_kernel(S=SEQ, D=D_MODEL, use_mask=False):
    """Emit the per-core Bass program. S = sequence len, D = model dim."""
    nc = bacc.Bacc(None, target_bir_lowering=False)
    EC = H_CORE * D_K                  # 256 q-proj cols per core
    ND = D // 128                      # d-model chunks
    NS = S // 128                      # seq tiles
    JW = min(1024, S)                  # scores psum width (banks*512)
    NJH = S // JW                      # psum tiles per i-tile row
    IW = min(512, S)                   # i-slice width (ctx moving N)
    NISL = S // IW                     # i-slices per head
    NT = IW // 128                     # i-tiles per slice
    NJB = S // 128                     # j blocks (transpose granularity)

    qT = nc.dram_tensor("qT", [D, S], BF16, kind="ExternalInput")
    kT = nc.dram_tensor("kT", [D, S], BF16, kind="ExternalInput")
    vT = nc.dram_tensor("vT", [D, S], BF16, kind="ExternalInput")
    wq = nc.dram_tensor("wq", [D, EC], BF16, kind="ExternalInput")
    wk = nc.dram_tensor("wk", [D, D_K], BF16, kind="ExternalInput")
    wv = nc.dram_tensor("wv", [D, D_K], BF16, kind="ExternalInput")
    wo = nc.dram_tensor("wo", [2, 128, D], BF16, kind="ExternalInput")
    cosT = nc.dram_tensor("cosT", [128, S], F32, kind="ExternalInput")
    sinT = nc.dram_tensor("sinT", [128, S], F32, kind="ExternalInput")
    permT = nc.dram_tensor("permT", [128, 128], F32, kind="ExternalInput")
    if use_mask:
        maskv = nc.dram_tensor("maskv", [S, S], F32, kind="ExternalInput")
    attn4 = nc.dram_tensor("attn4", [H_CORE, S, S], F32, kind="ExternalOutput")
    outp = nc.dram_tensor("outp", [S, D], F32, kind="ExternalOutput")

    with tile.TileContext(nc) as tc:
        # ---- persistent tiles ----
        const_pool = tc.tile_pool(name="const", bufs=1)
        qp_pool = tc.tile_pool(name="qp", bufs=1)
        with const_pool as cp, qp_pool as qpp:
            cos_sb = cp.tile([128, S], F32)
            sin_sb = cp.tile([128, S], F32)
            perm_sb = cp.tile([128, 128], F32)
            ident = cp.tile([128, 128], F32)
            identb = cp.tile([128, 128], BF16)
            nc.scalar.dma_start(cos_sb[:, :], cosT[:, :])
            nc.scalar.dma_start(sin_sb[:, :], sinT[:, :])
            nc.scalar.dma_start(perm_sb[:, :], permT[:, :])
            make_identity(nc, ident[:, :])
            make_identity(nc, identb[:, :])

            wk_sb = cp.tile([128, ND * D_K], BF16)
            wv_sb = cp.tile([128, ND * D_K], BF16)
            for d in range(ND):
                nc.scalar.dma_start(wk_sb[:, d * D_K:(d + 1) * D_K],
                                  wk[d * 128:(d + 1) * 128, :])
                nc.scalar.dma_start(wv_sb[:, d * D_K:(d + 1) * D_K],
                                  wv[d * 128:(d + 1) * 128, :])

            wo_sb = [cp.tile([128, D], BF16, tag=f"wo{ch}", name=f"wo{ch}")
                     for ch in range(2)]
            for ch in range(2):
                nc.scalar.dma_start(wo_sb[ch][:, :], wo[ch, :, :])

            # q'/k' (post-rope, transposed [c, s]) and v ([s, c]) and ctx^T
            qp_sb = [qpp.tile([128, S], BF16, tag=f"qp{e}", name=f"qp{e}")
                     for e in range(EC // 128)]
            kp_sb = qpp.tile([128, S], BF16)       # k' duplicated both halves
            v_sb = qpp.tile([128, NS * D_K], BF16)  # column block b = s-tile b
            ctxT_sb = [qpp.tile([128, S], BF16, tag=f"ctxT{e}", name=f"ctxT{e}")
                       for e in range(EC // 128)]

            # ================= Phase A: projections + rope =================
            with tc.tile_pool(name="wqp", bufs=1) as wqp, \
                 tc.tile_pool(name="stream", bufs=4) as stp, \
                 tc.tile_pool(name="araw", bufs=1) as arp, \
                 tc.tile_pool(name="scr", bufs=2) as scp, \
                 tc.tile_pool(name="psA", bufs=8, space="PSUM") as psA:
                wq_sb = wqp.tile([128, ND * EC], BF16)
                for d in range(ND):
                    nc.scalar.dma_start(wq_sb[:, d * EC:(d + 1) * EC],
                                      wq[d * 128:(d + 1) * 128, :])

                # ---- k projection: k_raw [64, S] ----
                k_raw = arp.tile([64, S], F32, tag="araw")
                psk = [psA.tile([64, 512], F32, tag="psA", name=f"psk{i}")
                       for i in range(S // 512)]
                kt_tiles = []
                for d in range(ND):
                    t = stp.tile([128, S], BF16, tag="stream")
                    nc.sync.dma_start(t[:, :], kT[d * 128:(d + 1) * 128, :])
                    kt_tiles.append(t)
                    for s4 in range(S // 512):
                        nc.tensor.matmul(
                            psk[s4][:, :],
                            r(wk_sb[:, d * D_K:(d + 1) * D_K]),
                            r(t[:, s4 * 512:(s4 + 1) * 512]),
                            start=(d == 0), stop=(d == ND - 1))
                for s4 in range(S // 512):
                    nc.vector.tensor_copy(k_raw[:, s4 * 512:(s4 + 1) * 512],
                                          psk[s4][:, :])
                # rope k into kp_sb[0:64], then duplicate to [64:128]
                for sl in range(S // 512):
                    sli = slice(sl * 512, (sl + 1) * 512)
                    rp = psA.tile([64, 512], F32, tag="psA")
                    nc.tensor.matmul(rp[:, :], perm_sb[0:64, 0:64],
                                     k_raw[:, sli], start=True, stop=True)
                    t1 = scp.tile([64, 512], F32, tag="scrk")
                    nc.vector.tensor_mul(t1[:, :], rp[:, :], sin_sb[0:64, sli])
                    t2 = scp.tile([64, 512], F32, tag="scrk2")
                    nc.vector.tensor_mul(t2[:, :], k_raw[:, sli], cos_sb[0:64, sli])
                    nc.vector.tensor_add(kp_sb[0:64, sli], t1[:, :], t2[:, :])
                nc.vector.tensor_copy(kp_sb[64:128, :], kp_sb[0:64, :])

                # ---- v projection: v_projT [64, S], then transpose to
                # v_sb [s-tile partitions, (b, c)] ----
                v_rawT = arp.tile([64, S], F32, tag="vrawT")
                psv = [psA.tile([64, 512], F32, tag="psA", name=f"psv{i}")
                       for i in range(S // 512)]
                for d in range(ND):
                    t = stp.tile([128, S], BF16, tag="stream")
                    nc.sync.dma_start(t[:, :], vT[d * 128:(d + 1) * 128, :])
                    for s4 in range(S // 512):
                        nc.tensor.matmul(
                            psv[s4][:, :],
                            r(wv_sb[:, d * D_K:(d + 1) * D_K]),
                            r(t[:, s4 * 512:(s4 + 1) * 512]),
                            start=(d == 0), stop=(d == ND - 1))
                for s4 in range(S // 512):
                    nc.vector.tensor_copy(v_rawT[:, s4 * 512:(s4 + 1) * 512],
                                          psv[s4][:, :])
                for g in range((NS * D_K + 511) // 512):
                    pvt = psA.tile([128, 512], F32, tag="psA")
                    nb = min(8, NS - g * 8)
                    for bb in range(nb):
                        b = g * 8 + bb
                        nc.tensor.transpose(
                            r(pvt[:, bb * D_K:(bb + 1) * D_K]),
                            r(v_rawT[:, b * 128:(b + 1) * 128]),
                            r(ident[0:64, 0:64]))
                    nc.vector.tensor_copy(
                        v_sb[:, g * 512:g * 512 + nb * D_K],
                        pvt[:, 0:nb * D_K])

                # ---- q projection: q_raw [e-tile][128, S] ----
                q_raw = [arp.tile([128, S], F32, tag=f"qraw{e}", name=f"qraw{e}")
                         for e in range(EC // 128)]
                psq = [[psA.tile([128, 512], F32, tag="psA", name=f"psq{e}_{i}")
                        for i in range(S // 512)] for e in range(EC // 128)]
                for d in range(ND):
                    t = stp.tile([128, S], BF16, tag="stream")
                    nc.sync.dma_start(t[:, :], qT[d * 128:(d + 1) * 128, :])
                    for e in range(EC // 128):
                        for s4 in range(S // 512):
                            nc.tensor.matmul(
                                psq[e][s4][:, :],
                                r(wq_sb[:, d * EC + e * 128:d * EC + (e + 1) * 128]),
                                r(t[:, s4 * 512:(s4 + 1) * 512]),
                                start=(d == 0), stop=(d == ND - 1))
                for e in range(EC // 128):
                    for s4 in range(S // 512):
                        nc.vector.tensor_copy(
                            q_raw[e][:, s4 * 512:(s4 + 1) * 512], psq[e][s4][:, :])
                # rope q
                for e in range(EC // 128):
                    for sl in range(S // 512):
                        sli = slice(sl * 512, (sl + 1) * 512)
                        rp = psA.tile([128, 512], F32, tag="psA")
                        nc.tensor.matmul(rp[:, :], perm_sb[:, :],
                                         q_raw[e][:, sli], start=True, stop=True)
                        t1 = scp.tile([128, 512], F32, tag="scrq")
                        nc.vector.tensor_mul(t1[:, :], rp[:, :], sin_sb[:, sli])
                        t2 = scp.tile([128, 512], F32, tag="scrq2")
                        nc.vector.tensor_mul(t2[:, :], q_raw[e][:, sli],
                                             cos_sb[:, sli])
                        nc.vector.tensor_add(qp_sb[e][:, sli], t1[:, :], t2[:, :])

            # ============ Phase B: scores / softmax / attn / ctx ============
            with tc.tile_pool(name="Epool", bufs=3) as ep, \
                 tc.tile_pool(name="Apool", bufs=NT + 1) as ap_, \
                 tc.tile_pool(name="Abpool", bufs=2 * NT + 1) as abp, \
                 tc.tile_pool(name="ETpool", bufs=NJB + 2) as etp, \
                 tc.tile_pool(name="rpool", bufs=2 * NT) as rp_, \
                 tc.tile_pool(name="opool", bufs=2) as op_, \
                 tc.tile_pool(name="mpool", bufs=2) as mp, \
                 tc.tile_pool(name="psS", bufs=2, space="PSUM") as psS, \
                 tc.tile_pool(name="psT", bufs=2, space="PSUM") as psT, \
                 tc.tile_pool(name="psC", bufs=2, space="PSUM") as psC:

                def transposes_and_ctx(h, isl, at_tiles):
                    # transpose attn[i-slice, :] -> ET [j, i-slice], then
                    # ctx^T [64, IW] = sum_jb v_b^T . ET_b
                    pb = (h % 2) * 64
                    et_tiles = []
                    for jb in range(NJB):
                        pt = psT.tile([128, IW], BF16, tag="psT")
                        for t in range(NT):
                            nc.tensor.transpose(
                                pt[:, t * 128:(t + 1) * 128],
                                at_tiles[t][:, jb * 128:(jb + 1) * 128],
                                identb[:, :])
                        et = etp.tile([128, IW], BF16, tag="ET")
                        nc.vector.tensor_copy(et[:, :], pt[:, :])
                        et_tiles.append(et)
                    pc = psC.tile([64, IW], F32, tag="psC")
                    for jb in range(NJB):
                        nc.tensor.matmul(
                            pc[:, :],
                            r(v_sb[:, jb * D_K:(jb + 1) * D_K]),
                            r(et_tiles[jb][:, :]),
                            start=(jb == 0), stop=(jb == NJB - 1))
                    nc.vector.tensor_copy(
                        ctxT_sb[h // 2][pb:pb + 64, isl * IW:(isl + 1) * IW],
                        pc[:, :])

                DW = min(512, D)

                def emit_out(isl):
                    # output projection for the s-tiles of this i-slice
                    for t in range(NT):
                        st = isl * NT + t
                        ot = op_.tile([128, D], F32, tag="out")
                        for dq in range(D // DW):
                            po = psT.tile([128, DW], F32, tag="psT")
                            for ch in range(2):
                                nc.tensor.matmul(
                                    po[:, :],
                                    r(ctxT_sb[ch][:, st * 128:(st + 1) * 128]),
                                    r(wo_sb[ch][:, dq * DW:(dq + 1) * DW]),
                                    start=(ch == 0), stop=(ch == 1))
                            nc.vector.tensor_copy(
                                ot[:, dq * DW:(dq + 1) * DW], po[:, :])
                        nc.gpsimd.dma_start(
                            outp[st * 128:(st + 1) * 128, :], ot[:, :])

                pending = None
                for isl in range(NISL):
                    for h in range(H_CORE):
                        qp = qp_sb[h // 2]
                        pb = (h % 2) * 64
                        at_tiles = []
                        for t in range(NT):
                            i0 = isl * IW + t * 128
                            pss = [psS.tile([128, JW], F32, tag="psS", name=f"pss{jh}")
                                   for jh in range(NJH)]
                            for jh in range(NJH):
                                for jq in range(JW // 512):
                                    j0 = jh * JW + jq * 512
                                    nc.tensor.matmul(
                                        pss[jh][:, jq * 512:(jq + 1) * 512],
                                        r(qp[pb:pb + 64, i0:i0 + 128]),
                                        r(kp_sb[pb:pb + 64, j0:j0 + 512]),
                                        start=True, stop=True)
                            if use_mask:
                                mt = mp.tile([128, S], F32, tag="mask")
                                nc.sync.dma_start(mt[:, :], maskv[i0:i0 + 128, :])
                                for jh in range(NJH):
                                    nc.vector.tensor_add(
                                        pss[jh][:, :], pss[jh][:, :],
                                        mt[:, jh * JW:(jh + 1) * JW])
                            Et = ep.tile([128, S], F32, tag="E")
                            racc = rp_.tile([128, NJH + 2], F32, tag="racc")
                            for jh in range(NJH):
                                nc.scalar.activation(
                                    Et[:, jh * JW:(jh + 1) * JW], pss[jh][:, :],
                                    AF.Exp, scale=0.125,
                                    accum_out=racc[:, jh:jh + 1])
                            # rowsum -> reciprocal
                            if NJH > 1:
                                nc.vector.tensor_add(
                                    racc[:, NJH:NJH + 1], racc[:, 0:1], racc[:, 1:2])
                                rsum = racc[:, NJH:NJH + 1]
                            else:
                                rsum = racc[:, 0:1]
                            nc.vector.reciprocal(racc[:, NJH + 1:NJH + 2], rsum)
                            At = ap_.tile([128, S], F32, tag="A")
                            Atb = abp.tile([128, S], BF16, tag="Ab")
                            # balance the two post-softmax passes across
                            # ScalarE/VectorE (ScalarE also owns exp)
                            if t % 2 == 0:
                                nc.vector.tensor_scalar_mul(
                                    At[:, :], Et[:, :],
                                    racc[:, NJH + 1:NJH + 2])
                                nc.scalar.activation(
                                    Atb[:, :], Et[:, :], AF.Copy,
                                    scale=racc[:, NJH + 1:NJH + 2])
                            else:
                                nc.scalar.activation(
                                    At[:, :], Et[:, :], AF.Copy,
                                    scale=racc[:, NJH + 1:NJH + 2])
                                nc.vector.tensor_scalar_mul(
                                    Atb[:, :], Et[:, :],
                                    racc[:, NJH + 1:NJH + 2])
                            nc.gpsimd.dma_start(attn4[h, i0:i0 + 128, :], At[:, :])
                            at_tiles.append(Atb)
                        if pending is not None:
                            transposes_and_ctx(*pending)
                        pending = (h, isl, at_tiles)
                    transposes_and_ctx(*pending)
                    pending = None
                    emit_out(isl)

    nc.compile()
    return nc


def _rope_tables(S):
    theta = 1.0 / (10000.0 ** (np.arange(0, D_K, 2, dtype=np.float32) / D_K))
    pos = np.arange(S, dtype=np.float32)
    freqs = np.outer(pos, theta)                       # (S, 32)
    cos = np.repeat(np.cos(freqs), 2, axis=-1)         # (S, 64)
    sin = np.repeat(np.sin(freqs), 2, axis=-1)
    cosT = np.ascontiguousarray(cos.T)                 # (64, S)
    sinT = np.ascontiguousarray(sin.T)
    return (np.tile(cosT, (2, 1)).astype(np.float32),
            np.tile(sinT, (2, 1)).astype(np.float32))  # (128, S)


def _perm_matrix():
    # lhsT for rotate_half: rot = P @ q, PT[c', c] = P[c, c']
    PT = np.zeros((64, 64), dtype=np.float32)
    idx = np.arange(32)
    PT[idx, idx + 32] = 1.0      # rot[c>=32] = q[c-32]
    PT[idx + 32, idx] = -1.0     # rot[c<32]  = -q[c+32]
    out = np.zeros((128, 128), dtype=np.float32)
    out[0:64, 0:64] = PT
    out[64:128, 64:128] = PT
    return out


_NC_CACHE = {}


def _get_nc(S, D, use_mask):
    key = (S, D, use_mask)
    if key not in _NC_CACHE:
        _NC_CACHE[key] = build_kernel(S, D, use_mask)
    return _NC_CACHE[key]


def kernel(query, key, value, mask, wq, wk, wv, wo, _trace=False):
    from concourse import bass_utils

    B, S, D = query.shape
    assert B == 1
    use_mask = bool(np.any(mask))

    nc = _get_nc(S, D, use_mask)

    import ml_dtypes
    bf16 = ml_dtypes.bfloat16
    qT = np.ascontiguousarray(np.asarray(query, np.float32)[0].T).astype(bf16)
    kT = np.ascontiguousarray(np.asarray(key, np.float32)[0].T).astype(bf16)
    vT = np.ascontiguousarray(np.asarray(value, np.float32)[0].T).astype(bf16)
    wq = np.asarray(wq, np.float32).astype(bf16)
    wk = np.asarray(wk, np.float32).astype(bf16)
    wv = np.asarray(wv, np.float32).astype(bf16)
    wo = np.asarray(wo, np.float32).astype(bf16)
    cosT, sinT = _rope_tables(S)
    permT = _perm_matrix()
    EC = H_CORE * D_K

    in_maps = []
    for c in range(N_CORES):
        m = {
            "qT": qT, "kT": kT, "vT": vT,
            "wq": np.ascontiguousarray(wq[:, c * EC:(c + 1) * EC]),
            "wk": np.ascontiguousarray(wk[:, c * D_K:(c + 1) * D_K]),
            "wv": np.ascontiguousarray(wv[:, c * D_K:(c + 1) * D_K]),
            "wo": np.ascontiguousarray(
                wo[c * EC:(c + 1) * EC, :].reshape(2, 128, D)),
            "cosT": cosT, "sinT": sinT, "permT": permT,
        }
        if use_mask:
            m["maskv"] = np.ascontiguousarray(
                (np.asarray(mask, np.float32)[0, 0] * np.float32(-8e9)))
        in_maps.append(m)

    res = bass_utils.run_bass_kernel_spmd(
        nc, in_maps, core_ids=list(range(N_CORES)), trace=_trace)

    out = np.zeros((S, D), dtype=np.float32)
    attn = np.empty((1, NUM_HEADS, S, S), dtype=np.float32)
    for c in range(N_CORES):
        out += res.results[c]["outp"]
        attn[0, c * H_CORE:(c + 1) * H_CORE] = res.results[c]["attn4"]
    if _trace:
        kernel._last_results = res
    return out[None], attn
